# revision 26
# baseline (speedup 1.0000x reference)
"""Bahdanau attention kernel for Trainium2 (Bass/Tile), data-parallel over batch.

v2: fine-grained 512-wide s-blocks, PE/DVE hybrid 5/3, fused DVE mul+reduce
(tensor_tensor_reduce), exp with fused z-accumulation, mask folded additively
into the PSUM energy accumulation, pm DMAs pre-issued on sync+gpsimd queues.
"""

import sys

if "/opt/trn_rl_repo" not in sys.path:
    sys.path.insert(0, "/opt/trn_rl_repo")

from contextlib import ExitStack

import numpy as np

import concourse.tile as tile
from concourse import bacc, masks, mybir
from concourse.bass_utils import run_bass_kernel_spmd

N_CORES = 8
B, S, Q = 64, 2048, 512
BL = B // N_CORES          # local batches per core
QC = Q // 128              # 128-wide q chunks
SB = 512                   # s-block width
NSB = S // SB              # 4 blocks
ST = SB // 128             # 128-row s sub-tiles per block

PE_B = 5                   # batches on the PE (transpose+matmul) path
DVE_B = BL - PE_B          # batches on the DVE (add+tanh+ttr) path
WP = 2 * PE_B - 1          # padded we width per q-chunk

PM_BUFS = 11               # pm tile ring (1MB each)

# Use only baseline-proven ops (no activation accum_out, no
# tensor_tensor_reduce, no scalar-engine scale-by-AP) to bisect HW failures.
SAFE_OPS = True

F32 = mybir.dt.float32
F32R = mybir.dt.float32r
I32 = mybir.dt.int32

_CACHE = {}


def _build():
    nc = bacc.Bacc(
        "TRN2",
        target_bir_lowering=False,
        debug=False,
        enable_asserts=False,
        num_devices=N_CORES,
    )
    pm_d = nc.dram_tensor("pm", [BL, S, Q], F32R, kind="ExternalInput").ap()
    q_d = nc.dram_tensor("q", [BL, Q], F32, kind="ExternalInput").ap()
    mask_d = nc.dram_tensor("mask", [BL, S], I32, kind="ExternalInput").ap()
    wq_d = nc.dram_tensor("wq", [Q, Q], F32, kind="ExternalInput").ap()
    we_d = nc.dram_tensor("we", [Q], F32, kind="ExternalInput").ap()
    attn_d = nc.dram_tensor("attn", [BL, S], F32, kind="ExternalOutput").ap()

    tanh = mybir.ActivationFunctionType.Tanh
    exp = mybir.ActivationFunctionType.Exp

    with tile.TileContext(nc) as tc, ExitStack() as ctx:
        const = ctx.enter_context(tc.tile_pool(name="const", bufs=1))
        setup = ctx.enter_context(tc.tile_pool(name="setup", bufs=1))
        pmp = ctx.enter_context(tc.tile_pool(name="pmp", bufs=PM_BUFS))
        thp = ctx.enter_context(tc.tile_pool(name="thp", bufs=3))
        ptp = ctx.enter_context(tc.tile_pool(name="ptp", bufs=4, space="PSUM"))
        ep = ctx.enter_context(tc.tile_pool(name="ep", bufs=2, space="PSUM"))
        ecp = ctx.enter_context(tc.tile_pool(name="ecp", bufs=2, space="PSUM"))
        outp = ctx.enter_context(tc.tile_pool(name="outp", bufs=1))

        # ---- constants FIRST (ident uses gpsimd; must precede pm issues) --
        ident = const.tile([128, 128], F32)
        masks.make_identity(nc, ident[:])
        ident_r = const.tile([128, 128], F32R)
        nc.vector.tensor_copy(ident_r[:], ident[:])
        ones_f = setup.tile([1, 128], F32)
        nc.vector.memset(ones_f[:], 1.0)
        ones_r = setup.tile([1, 128], F32R)
        nc.vector.tensor_copy(ones_r[:], ones_f[:])

        # ---- setup DMAs on scalar/vector queues (sync+gpsimd carry pm) ----
        wq_nat = setup.tile([128, QC * Q], F32)
        nc.scalar.dma_start(
            wq_nat[:].rearrange("p (c q) -> p c q", c=QC),
            wq_d.rearrange("(c p) q -> p c q", p=128),
        )
        q_nat = setup.tile([BL, Q], F32)
        nc.scalar.dma_start(q_nat[:], q_d[:])
        weT = setup.tile([128, QC], F32)
        nc.scalar.dma_start(weT[:], we_d.rearrange("(c p) -> p c", p=128))
        we_row = setup.tile([1, Q], F32)
        nc.scalar.dma_start(we_row[:], we_d.rearrange("(o q) -> o q", o=1))
        mask_i = setup.tile([BL, S], I32)
        nc.scalar.dma_start(mask_i[:], mask_d[:])

        # ---- pm DMA issue, alternating sync / gpsimd queues ---------------
        # Bounded lookahead: blocks 0-1 issued up front, block sb+1 issued at
        # the top of block sb (a fully pre-issued queue wedges the HW DGE).
        pm_tiles = {}

        def issue_pm_block(sb):
            for b in range(BL):
                pm_t = pmp.tile([128, ST * Q], F32R, tag="pm",
                                name=f"pm_{sb}_{b}")
                eng = nc.sync if (sb * BL + b) % 2 == 0 else nc.gpsimd
                s0 = sb * SB
                eng.dma_start(
                    pm_t[:].rearrange("p (t q) -> p t q", t=ST),
                    pm_d[b, s0 : s0 + SB, :].rearrange("(t p) q -> p t q",
                                                       p=128),
                )
                pm_tiles[(sb, b)] = pm_t

        issue_pm_block(0)
        issue_pm_block(1)

        # mask penalty: (mask - 1) * 1000  (0 where mask==1, -1000 where 0)
        maskpen = setup.tile([BL, S], F32R)
        nc.vector.tensor_scalar(
            maskpen[:], mask_i[:], 1000.0, -1000.0,
            op0=mybir.AluOpType.mult, op1=mybir.AluOpType.add,
        )
        # DVE-path rows of the penalty re-based to partition 0 (compute
        # engines cannot address partition bases other than 0/32/64)
        maskpen_dve = setup.tile([DVE_B, S], F32)
        nc.scalar.dma_start(maskpen_dve[:], maskpen[PE_B:BL, :].bitcast(F32))

        # we_bc [128, Q] broadcast of we across partitions (for DVE ttr)
        we_row_r = setup.tile([1, Q], F32R)
        nc.vector.tensor_copy(we_row_r[:], we_row[:])
        bc_ps = ptp.tile([128, Q], F32, tag="pt", name="bc_we")
        nc.tensor.matmul(bc_ps[:], ones_r[:], we_row_r[:], start=True, stop=True)
        we_bc = setup.tile([128, Q], F32)
        nc.vector.tensor_copy(we_bc[:], bc_ps[:])

        # we_mm [128, QC*WP]: padded we so matmul lands on out partition b
        we_pad = setup.tile([128, QC * WP], F32)
        nc.vector.memset(we_pad[:], 0.0)
        for qc in range(QC):
            nc.vector.tensor_copy(
                we_pad[:, qc * WP + PE_B - 1 : qc * WP + PE_B],
                weT[:, qc : qc + 1],
            )
        we_mm = setup.tile([128, QC * WP], F32R)
        nc.vector.tensor_copy(we_mm[:], we_pad[:])

        # ---- transpose Wq and query so q lands on partitions --------------
        wqT = setup.tile([128, QC * Q], F32)
        for qc in range(QC):
            for c in range(QC):
                pt = ptp.tile([128, 128], F32, tag="pt", name=f"wt_{qc}_{c}")
                nc.tensor.transpose(
                    pt[:], wq_nat[:, c * Q + qc * 128 : c * Q + (qc + 1) * 128],
                    ident[:],
                )
                nc.vector.tensor_copy(
                    wqT[:, qc * Q + c * 128 : qc * Q + (c + 1) * 128], pt[:]
                )
        qT = setup.tile([128, QC * BL], F32)
        for qc in range(QC):
            pt = ptp.tile([128, BL], F32, tag="pt", name=f"qt_{qc}")
            nc.tensor.transpose(
                pt[:], q_nat[:, qc * 128 : (qc + 1) * 128], ident[0:BL, 0:BL]
            )
            nc.vector.tensor_copy(qT[:, qc * BL : (qc + 1) * BL], pt[:])

        # ---- pqT: projected query, q on partitions ------------------------
        pqT = setup.tile([128, QC * BL], F32)
        for dc in range(QC):
            acc = ep.tile([128, BL], F32, tag="e", name=f"pq_{dc}")
            for qc in range(QC):
                nc.tensor.matmul(
                    acc[:],
                    wqT[:, qc * Q + dc * 128 : qc * Q + (dc + 1) * 128],
                    qT[:, qc * BL : (qc + 1) * BL],
                    start=(qc == 0),
                    stop=(qc == QC - 1),
                )
            nc.vector.tensor_copy(pqT[:, dc * BL : (dc + 1) * BL], acc[:])

        # ---- pq broadcast tiles for DVE-path batches ----------------------
        pq_bc = {}
        pq_row = setup.tile([1, Q], F32R, name="pqrow")
        for b in range(PE_B, BL):
            row_ps = ptp.tile([1, Q], F32, tag="pt", name=f"rps_{b}")
            for qc in range(QC):
                nc.tensor.transpose(
                    row_ps[:, qc * 128 : (qc + 1) * 128],
                    pqT[:, qc * BL + b : qc * BL + b + 1],
                    ident[:],
                )
            nc.vector.tensor_copy(pq_row[:], row_ps[:])
            bc2 = ptp.tile([128, Q], F32, tag="pt", name=f"bc_{b}")
            nc.tensor.matmul(bc2[:], ones_r[:], pq_row[:], start=True, stop=True)
            t_bc = setup.tile([128, Q], F32, name=f"pqbc_{b}")
            nc.vector.tensor_copy(t_bc[:], bc2[:])
            pq_bc[b] = t_bc

        # ---- persistent main-loop state -----------------------------------
        # PE-path rows live on partitions 0..PE_B-1; DVE-path rows on their
        # own partition-0-based tiles (partition bases must be 0/32/64).
        p_e = outp.tile([PE_B, S], F32)        # exp(masked energy), PE rows
        p_dve = outp.tile([DVE_B, S], F32)     # exp(masked energy), DVE rows
        e_dve = outp.tile([DVE_B, S], F32)     # raw DVE-row energies
        z_pe_part = outp.tile([PE_B, NSB], F32)
        z_dve_part = outp.tile([DVE_B, NSB], F32)

        def make_dve_finish(sb):
            # mask-add + exp for DVE rows of block sb; call em_fn (vector)
            # first, then exp_fn (scalar) — em must be emitted before exp.
            em = thp.tile([DVE_B, SB], F32, tag="em", bufs=2, name=f"em_{sb}")
            cols = slice(sb * SB, (sb + 1) * SB)

            def em_fn():
                nc.vector.tensor_add(
                    em[:], e_dve[:, cols], maskpen_dve[:, cols]
                )

            def exp_fn():
                if SAFE_OPS:
                    nc.scalar.activation(p_dve[:, cols], em[:], exp)
                    nc.vector.tensor_reduce(
                        z_dve_part[:, sb : sb + 1], p_dve[:, cols],
                        axis=mybir.AxisListType.X, op=mybir.AluOpType.add,
                    )
                else:
                    nc.scalar.activation(
                        p_dve[:, cols], em[:], exp,
                        accum_out=z_dve_part[:, sb : sb + 1],
                    )

            return em_fn, exp_fn

        pending = None  # (em_fn, exp_fn) from previous block

        # ---- main loop ----------------------------------------------------
        for sb in range(NSB):
            if sb + 2 < NSB:
                issue_pm_block(sb + 2)
            e_ps = ep.tile([PE_B, SB], F32, tag="e", name=f"e_{sb}")
            # mask penalty folded additively into the accumulation (start)
            nc.tensor.matmul(
                e_ps[:],
                ident_r[0:BL, 0:PE_B],
                maskpen[:, sb * SB : (sb + 1) * SB],
                start=True,
                stop=False,
            )

            pe_units = [("pe", b, qc) for b in range(PE_B) for qc in range(QC)]
            dve_units = [("dve", b, t) for b in range(PE_B, BL)
                         for t in range(ST)]
            dve_scale = 0.8 if sb == NSB - 1 else 1.0
            keyed = [((i + 0.5) / len(pe_units), u)
                     for i, u in enumerate(pe_units)]
            keyed += [(dve_scale * (i + 0.5) / max(1, len(dve_units)), u)
                      for i, u in enumerate(dve_units)]
            units = [u for _, u in sorted(keyed, key=lambda x: x[0])]

            ecols = {}
            for b in range(PE_B, BL):
                ecols[b] = thp.tile([128, ST], F32, tag=f"ecol{b - PE_B}",
                                    bufs=2, name=f"ec_{b}_{sb}")

            n_pe_done = 0
            for ui, (kind, b, j) in enumerate(units):
                if pending is not None and ui == 3:
                    pending[0]()          # em add (vector)
                if pending is not None and ui == 6:
                    pending[1]()          # exp (scalar)
                    pending = None
                pm_t = pm_tiles[(sb, b)]
                if kind == "pe":
                    qc = j
                    pt = ptp.tile([128, SB], F32R, tag="pt",
                                  name=f"pt_{b}_{sb}_{qc}")
                    for t in range(ST):
                        nc.tensor.transpose(
                            pt[:, t * 128 : (t + 1) * 128],
                            pm_t[:, t * Q + qc * 128 : t * Q + (qc + 1) * 128],
                            ident_r[:],
                        )
                    th = thp.tile([128, SB], F32R, tag="th", bufs=3,
                                  name=f"th_{b}_{sb}_{qc}")
                    nc.scalar.activation(
                        th[:], pt[:], tanh,
                        bias=pqT[:, qc * BL + b : qc * BL + b + 1], scale=1.0,
                    )
                    n_pe_done += 1
                    nc.tensor.matmul(
                        e_ps[:],
                        we_mm[:, qc * WP + PE_B - 1 - b
                              : qc * WP + 2 * PE_B - 1 - b],
                        th[:],
                        start=False,
                        stop=(n_pe_done == len(pe_units)),
                    )
                else:
                    t = j
                    ta = thp.tile([128, Q], F32, tag="ta", bufs=3,
                                  name=f"ta_{b}_{sb}_{t}")
                    nc.vector.tensor_add(
                        ta[:], pm_t[:, t * Q : (t + 1) * Q].bitcast(F32),
                        pq_bc[b][:],
                    )
                    tt = thp.tile([128, Q], F32, tag="tt", bufs=3,
                                  name=f"tt_{b}_{sb}_{t}")
                    nc.scalar.activation(tt[:], ta[:], tanh)
                    sc = thp.tile([128, Q], F32, tag="sc", bufs=2,
                                  name=f"sc_{b}_{sb}_{t}")
                    if SAFE_OPS:
                        nc.vector.tensor_mul(sc[:], tt[:], we_bc[:])
                        nc.vector.tensor_reduce(
                            ecols[b][:, t : t + 1], sc[:],
                            axis=mybir.AxisListType.X, op=mybir.AluOpType.add,
                        )
                    else:
                        nc.vector.tensor_tensor_reduce(
                            out=sc[:],
                            in0=tt[:],
                            in1=we_bc[:],
                            scale=1.0,
                            scalar=0.0,
                            op0=mybir.AluOpType.mult,
                            op1=mybir.AluOpType.add,
                            accum_out=ecols[b][:, t : t + 1],
                        )

            # DVE rows: gather energies into rows of eraw via SBUF->SBUF DMA
            ecps = ecp.tile([ST, DVE_B * 128], F32, tag="ec",
                            name=f"ecp_{sb}")
            for b in range(PE_B, BL):
                i = b - PE_B
                nc.tensor.transpose(
                    ecps[:, i * 128 : (i + 1) * 128], ecols[b][:], ident[:]
                )
            ecT = thp.tile([ST, DVE_B * 128], F32, tag="ecT", bufs=2,
                           name=f"ecT_{sb}")
            nc.vector.tensor_copy(ecT[:], ecps[:])
            for i in range(DVE_B):
                nc.scalar.dma_start(
                    e_dve[i : i + 1, sb * SB : (sb + 1) * SB],
                    ecT[:, i * 128 : (i + 1) * 128],
                )

            em_fn, exp_fn = make_dve_finish(sb)
            if sb == NSB - 1:
                em_fn()
                exp_fn()
            else:
                pending = (em_fn, exp_fn)

            # PE rows: exp straight out of PSUM with fused z accumulation
            if SAFE_OPS:
                nc.scalar.activation(
                    p_e[:, sb * SB : (sb + 1) * SB], e_ps[:], exp,
                )
                nc.vector.tensor_reduce(
                    z_pe_part[:, sb : sb + 1],
                    p_e[:, sb * SB : (sb + 1) * SB],
                    axis=mybir.AxisListType.X, op=mybir.AluOpType.add,
                )
            else:
                nc.scalar.activation(
                    p_e[:, sb * SB : (sb + 1) * SB], e_ps[:], exp,
                    accum_out=z_pe_part[:, sb : sb + 1],
                )

        # ---- finish softmax (per path: partition bases must be 0) ---------
        z_pe = outp.tile([PE_B, 1], F32)
        nc.vector.tensor_reduce(z_pe[:], z_pe_part[:],
                                axis=mybir.AxisListType.X,
                                op=mybir.AluOpType.add)
        zr_pe = outp.tile([PE_B, 1], F32)
        nc.vector.reciprocal(zr_pe[:], z_pe[:])
        z_dve = outp.tile([DVE_B, 1], F32)
        nc.vector.tensor_reduce(z_dve[:], z_dve_part[:],
                                axis=mybir.AxisListType.X,
                                op=mybir.AluOpType.add)
        zr_dve = outp.tile([DVE_B, 1], F32)
        nc.vector.reciprocal(zr_dve[:], z_dve[:])
        # reuse dead tiles as output staging: mask_i (PE rows), e_dve (DVE)
        a_pe = mask_i[0:PE_B, :].bitcast(F32)
        a_dve = e_dve
        for h in range(2):
            hs = S // 2
            cols = slice(h * hs, (h + 1) * hs)
            # PE rows scaled on scalar engine, DVE rows on vector — parallel
            if SAFE_OPS:
                nc.vector.tensor_scalar(
                    a_pe[:, cols], p_e[:, cols], zr_pe[:], None,
                    op0=mybir.AluOpType.mult,
                )
            else:
                nc.scalar.mul(a_pe[:, cols], p_e[:, cols], zr_pe[:])
            nc.sync.dma_start(attn_d[0:PE_B, cols], a_pe[:, cols])
            nc.vector.tensor_scalar(
                a_dve[:, cols], p_dve[:, cols], zr_dve[:], None,
                op0=mybir.AluOpType.mult,
            )
            nc.gpsimd.dma_start(attn_d[PE_B:BL, cols], a_dve[:, cols])

    nc.compile()
    return nc


def _get_nc():
    if "nc" not in _CACHE:
        _CACHE["nc"] = _build()
    return _CACHE["nc"]


def _make_in_maps(query, projected_memory, mask, Wq, We):
    query = np.asarray(query, dtype=np.float32)
    pm = np.asarray(projected_memory, dtype=np.float32)
    mask = np.asarray(mask, dtype=np.int32)
    wq = np.ascontiguousarray(np.asarray(Wq, dtype=np.float32))
    we = np.ascontiguousarray(np.asarray(We, dtype=np.float32))
    in_maps = []
    for i in range(N_CORES):
        lo, hi = i * BL, (i + 1) * BL
        in_maps.append(
            {
                "pm": np.ascontiguousarray(pm[lo:hi]),
                "q": np.ascontiguousarray(query[0, lo:hi, :]),
                "mask": np.ascontiguousarray(mask[lo:hi]),
                "wq": wq,
                "we": we,
            }
        )
    return in_maps


def run_spmd(query, projected_memory, mask, Wq, We, **spmd_kwargs):
    nc = _get_nc()
    in_maps = _make_in_maps(query, projected_memory, mask, Wq, We)
    return run_bass_kernel_spmd(nc, in_maps, list(range(N_CORES)), **spmd_kwargs)


def kernel(query, projected_memory, mask, Wq, We):
    res = run_spmd(query, projected_memory, mask, Wq, We)
    attn = np.concatenate([res.results[i]["attn"] for i in range(N_CORES)], axis=0)
    return attn[:, None, :].astype(np.float32)


# revision 36
# speedup vs baseline: 1.0796x; 1.0796x over previous
"""Bahdanau attention kernel for Trainium2 (Bass/Tile), data-parallel over batch.

v2: fine-grained 512-wide s-blocks, PE/DVE hybrid 5/3, fused DVE mul+reduce
(tensor_tensor_reduce), exp with fused z-accumulation, mask folded additively
into the PSUM energy accumulation, pm DMAs pre-issued on sync+gpsimd queues.
"""

import sys

if "/opt/trn_rl_repo" not in sys.path:
    sys.path.insert(0, "/opt/trn_rl_repo")

from contextlib import ExitStack

import numpy as np

import concourse.tile as tile
from concourse import bacc, masks, mybir
from concourse.bass_utils import run_bass_kernel_spmd

N_CORES = 8
B, S, Q = 64, 2048, 512
BL = B // N_CORES          # local batches per core
QC = Q // 128              # 128-wide q chunks
SB = 512                   # s-block width
NSB = S // SB              # 4 blocks
ST = SB // 128             # 128-row s sub-tiles per block

PE_B = 5                   # batches on the PE (transpose+matmul) path
DVE_B = BL - PE_B          # batches on the DVE (add+tanh+ttr) path
WP = 2 * PE_B - 1          # padded we width per q-chunk

PM_BUFS = 11               # pm tile ring (1MB each)

# Feature flags for ops that passed CoreSim but may misbehave on HW
# (bisected individually; all-off config passed HW at 169us).
USE_TTR = False     # DVE tensor_tensor_reduce — CONFIRMED BREAKS HW
USE_ACCUM = False   # activation accum_out (fused z row-sum)
USE_SMUL = False    # scalar-engine activation Copy with AP scale
GP_MUL = True       # DVE-path multiply on gpsimd (Pool) instead of vector

F32 = mybir.dt.float32
F32R = mybir.dt.float32r
I32 = mybir.dt.int32

_CACHE = {}


def _build():
    nc = bacc.Bacc(
        "TRN2",
        target_bir_lowering=False,
        debug=False,
        enable_asserts=False,
        num_devices=N_CORES,
    )
    pm_d = nc.dram_tensor("pm", [BL, S, Q], F32R, kind="ExternalInput").ap()
    q_d = nc.dram_tensor("q", [BL, Q], F32, kind="ExternalInput").ap()
    mask_d = nc.dram_tensor("mask", [BL, S], I32, kind="ExternalInput").ap()
    wq_d = nc.dram_tensor("wq", [Q, Q], F32, kind="ExternalInput").ap()
    we_d = nc.dram_tensor("we", [Q], F32, kind="ExternalInput").ap()
    attn_d = nc.dram_tensor("attn", [BL, S], F32, kind="ExternalOutput").ap()

    tanh = mybir.ActivationFunctionType.Tanh
    exp = mybir.ActivationFunctionType.Exp

    with tile.TileContext(nc) as tc, ExitStack() as ctx:
        const = ctx.enter_context(tc.tile_pool(name="const", bufs=1))
        setup = ctx.enter_context(tc.tile_pool(name="setup", bufs=1))
        pmp = ctx.enter_context(tc.tile_pool(name="pmp", bufs=PM_BUFS))
        thp = ctx.enter_context(tc.tile_pool(name="thp", bufs=3))
        ptp = ctx.enter_context(tc.tile_pool(name="ptp", bufs=4, space="PSUM"))
        ep = ctx.enter_context(tc.tile_pool(name="ep", bufs=2, space="PSUM"))
        ecp = ctx.enter_context(tc.tile_pool(name="ecp", bufs=2, space="PSUM"))
        outp = ctx.enter_context(tc.tile_pool(name="outp", bufs=1))

        # ---- constants FIRST (ident uses gpsimd; must precede pm issues) --
        ident = const.tile([128, 128], F32)
        masks.make_identity(nc, ident[:])
        ident_r = const.tile([128, 128], F32R)
        nc.vector.tensor_copy(ident_r[:], ident[:])
        ones_f = setup.tile([1, 128], F32)
        nc.vector.memset(ones_f[:], 1.0)
        ones_r = setup.tile([1, 128], F32R)
        nc.vector.tensor_copy(ones_r[:], ones_f[:])

        # ---- setup DMAs FIRST on the sync queue (ahead of the pm storm;
        # a separate queue gets starved behind 16MB of pm descriptors) ------
        wq_nat = setup.tile([128, QC * Q], F32)
        nc.sync.dma_start(
            wq_nat[:].rearrange("p (c q) -> p c q", c=QC),
            wq_d.rearrange("(c p) q -> p c q", p=128),
        )
        q_nat = setup.tile([BL, Q], F32)
        nc.sync.dma_start(q_nat[:], q_d[:])
        weT = setup.tile([128, QC], F32)
        nc.sync.dma_start(weT[:], we_d.rearrange("(c p) -> p c", p=128))
        we_row = setup.tile([1, Q], F32)
        nc.sync.dma_start(we_row[:], we_d.rearrange("(o q) -> o q", o=1))
        mask_i = setup.tile([BL, S], I32)
        nc.sync.dma_start(mask_i[:], mask_d[:])

        # ---- pm DMA issue, alternating sync / gpsimd queues ---------------
        # Bounded lookahead: blocks 0-1 issued up front, block sb+1 issued at
        # the top of block sb (a fully pre-issued queue wedges the HW DGE).
        pm_tiles = {}

        def issue_pm_block(sb, split):
            # split=True alternates sync/gpsimd for a fast ramp; only safe
            # when no buffer-wait can occur (gpsimd later carries muls and
            # must never block at its queue head).
            for b in range(BL):
                pm_t = pmp.tile([128, ST * Q], F32R, tag="pm",
                                name=f"pm_{sb}_{b}")
                eng = nc.gpsimd if (split and b % 2 == 1) else nc.sync
                s0 = sb * SB
                eng.dma_start(
                    pm_t[:].rearrange("p (t q) -> p t q", t=ST),
                    pm_d[b, s0 : s0 + SB, :].rearrange("(t p) q -> p t q",
                                                       p=128),
                )
                pm_tiles[(sb, b)] = pm_t

        issue_pm_block(0, split=True)
        issue_pm_block(1, split=False)

        # mask penalty: (mask - 1) * 1000  (0 where mask==1, -1000 where 0)
        maskpen = setup.tile([BL, S], F32R)
        nc.vector.tensor_scalar(
            maskpen[:], mask_i[:], 1000.0, -1000.0,
            op0=mybir.AluOpType.mult, op1=mybir.AluOpType.add,
        )
        # DVE-path rows of the penalty re-based to partition 0 (compute
        # engines cannot address partition bases other than 0/32/64)
        maskpen_dve = setup.tile([DVE_B, S], F32)
        nc.scalar.dma_start(maskpen_dve[:], maskpen[PE_B:BL, :].bitcast(F32))

        # we_bc [128, Q] broadcast of we across partitions (for DVE ttr)
        we_row_r = setup.tile([1, Q], F32R)
        nc.vector.tensor_copy(we_row_r[:], we_row[:])
        bc_ps = ptp.tile([128, Q], F32, tag="pt", name="bc_we")
        nc.tensor.matmul(bc_ps[:], ones_r[:], we_row_r[:], start=True, stop=True)
        we_bc = setup.tile([128, Q], F32)
        nc.vector.tensor_copy(we_bc[:], bc_ps[:])

        # we_mm [128, QC*WP]: padded we so matmul lands on out partition b
        we_pad = setup.tile([128, QC * WP], F32)
        nc.vector.memset(we_pad[:], 0.0)
        for qc in range(QC):
            nc.vector.tensor_copy(
                we_pad[:, qc * WP + PE_B - 1 : qc * WP + PE_B],
                weT[:, qc : qc + 1],
            )
        we_mm = setup.tile([128, QC * WP], F32R)
        nc.vector.tensor_copy(we_mm[:], we_pad[:])

        # ---- transpose Wq and query so q lands on partitions --------------
        wqT = setup.tile([128, QC * Q], F32)
        for qc in range(QC):
            for c in range(QC):
                pt = ptp.tile([128, 128], F32, tag="pt", name=f"wt_{qc}_{c}")
                nc.tensor.transpose(
                    pt[:], wq_nat[:, c * Q + qc * 128 : c * Q + (qc + 1) * 128],
                    ident[:],
                )
                nc.vector.tensor_copy(
                    wqT[:, qc * Q + c * 128 : qc * Q + (c + 1) * 128], pt[:]
                )
        qT = setup.tile([128, QC * BL], F32)
        for qc in range(QC):
            pt = ptp.tile([128, BL], F32, tag="pt", name=f"qt_{qc}")
            nc.tensor.transpose(
                pt[:], q_nat[:, qc * 128 : (qc + 1) * 128], ident[0:BL, 0:BL]
            )
            nc.vector.tensor_copy(qT[:, qc * BL : (qc + 1) * BL], pt[:])

        # ---- pqT: projected query, q on partitions ------------------------
        pqT = setup.tile([128, QC * BL], F32)
        for dc in range(QC):
            acc = ep.tile([128, BL], F32, tag="e", name=f"pq_{dc}")
            for qc in range(QC):
                nc.tensor.matmul(
                    acc[:],
                    wqT[:, qc * Q + dc * 128 : qc * Q + (dc + 1) * 128],
                    qT[:, qc * BL : (qc + 1) * BL],
                    start=(qc == 0),
                    stop=(qc == QC - 1),
                )
            nc.vector.tensor_copy(pqT[:, dc * BL : (dc + 1) * BL], acc[:])

        # ---- pq broadcast tiles for DVE-path batches ----------------------
        pq_bc = {}
        pq_row = setup.tile([1, Q], F32R, name="pqrow")
        for b in range(PE_B, BL):
            row_ps = ptp.tile([1, Q], F32, tag="pt", name=f"rps_{b}")
            for qc in range(QC):
                nc.tensor.transpose(
                    row_ps[:, qc * 128 : (qc + 1) * 128],
                    pqT[:, qc * BL + b : qc * BL + b + 1],
                    ident[:],
                )
            nc.vector.tensor_copy(pq_row[:], row_ps[:])
            bc2 = ptp.tile([128, Q], F32, tag="pt", name=f"bc_{b}")
            nc.tensor.matmul(bc2[:], ones_r[:], pq_row[:], start=True, stop=True)
            t_bc = setup.tile([128, Q], F32, name=f"pqbc_{b}")
            nc.vector.tensor_copy(t_bc[:], bc2[:])
            pq_bc[b] = t_bc

        # ---- persistent main-loop state -----------------------------------
        # PE-path rows live on partitions 0..PE_B-1; DVE-path rows on their
        # own partition-0-based tiles (partition bases must be 0/32/64).
        p_e = outp.tile([PE_B, S], F32)        # exp(masked energy), PE rows
        p_dve = outp.tile([DVE_B, S], F32)     # exp(masked energy), DVE rows
        e_dve = outp.tile([DVE_B, S], F32)     # raw DVE-row energies
        z_pe_part = outp.tile([PE_B, NSB], F32)
        z_dve_part = outp.tile([DVE_B, NSB], F32)

        def make_dve_finish(sb):
            # mask-add + exp for DVE rows of block sb; call em_fn (vector)
            # first, then exp_fn (scalar) — em must be emitted before exp.
            em = thp.tile([DVE_B, SB], F32, tag="em", bufs=2, name=f"em_{sb}")
            cols = slice(sb * SB, (sb + 1) * SB)

            def em_fn():
                nc.vector.tensor_add(
                    em[:], e_dve[:, cols], maskpen_dve[:, cols]
                )

            def exp_fn():
                if not USE_ACCUM:
                    nc.scalar.activation(p_dve[:, cols], em[:], exp)
                    nc.vector.tensor_reduce(
                        z_dve_part[:, sb : sb + 1], p_dve[:, cols],
                        axis=mybir.AxisListType.X, op=mybir.AluOpType.add,
                    )
                else:
                    nc.scalar.activation(
                        p_dve[:, cols], em[:], exp,
                        accum_out=z_dve_part[:, sb : sb + 1],
                    )

            return em_fn, exp_fn

        pending = None  # (em_fn, exp_fn) from previous block

        # ---- main loop ----------------------------------------------------
        for sb in range(NSB):
            if sb + 2 < NSB:
                issue_pm_block(sb + 2, split=False)
            e_ps = ep.tile([PE_B, SB], F32, tag="e", name=f"e_{sb}")
            # mask penalty folded additively into the accumulation (start)
            nc.tensor.matmul(
                e_ps[:],
                ident_r[0:BL, 0:PE_B],
                maskpen[:, sb * SB : (sb + 1) * SB],
                start=True,
                stop=False,
            )

            pe_units = [("pe", b, qc) for b in range(PE_B) for qc in range(QC)]
            dve_units = [("dve", b, t) for b in range(PE_B, BL)
                         for t in range(ST)]
            dve_scale = 0.8 if sb == NSB - 1 else 1.0
            keyed = [((i + 0.5) / len(pe_units), u)
                     for i, u in enumerate(pe_units)]
            keyed += [(dve_scale * (i + 0.5) / max(1, len(dve_units)), u)
                      for i, u in enumerate(dve_units)]
            units = [u for _, u in sorted(keyed, key=lambda x: x[0])]

            ecols = {}
            for b in range(PE_B, BL):
                ecols[b] = thp.tile([128, ST], F32, tag=f"ecol{b - PE_B}",
                                    bufs=2, name=f"ec_{b}_{sb}")

            n_pe_done = 0
            for ui, (kind, b, j) in enumerate(units):
                if pending is not None and ui == 3:
                    pending[0]()          # em add (vector)
                if pending is not None and ui == 6:
                    pending[1]()          # exp (scalar)
                    pending = None
                pm_t = pm_tiles[(sb, b)]
                if kind == "pe":
                    qc = j
                    pt = ptp.tile([128, SB], F32R, tag="pt",
                                  name=f"pt_{b}_{sb}_{qc}")
                    for t in range(ST):
                        nc.tensor.transpose(
                            pt[:, t * 128 : (t + 1) * 128],
                            pm_t[:, t * Q + qc * 128 : t * Q + (qc + 1) * 128],
                            ident_r[:],
                        )
                    th = thp.tile([128, SB], F32R, tag="th", bufs=3,
                                  name=f"th_{b}_{sb}_{qc}")
                    nc.scalar.activation(
                        th[:], pt[:], tanh,
                        bias=pqT[:, qc * BL + b : qc * BL + b + 1], scale=1.0,
                    )
                    n_pe_done += 1
                    nc.tensor.matmul(
                        e_ps[:],
                        we_mm[:, qc * WP + PE_B - 1 - b
                              : qc * WP + 2 * PE_B - 1 - b],
                        th[:],
                        start=False,
                        stop=(n_pe_done == len(pe_units)),
                    )
                else:
                    t = j
                    ta = thp.tile([128, Q], F32, tag="ta", bufs=3,
                                  name=f"ta_{b}_{sb}_{t}")
                    nc.vector.tensor_add(
                        ta[:], pm_t[:, t * Q : (t + 1) * Q].bitcast(F32),
                        pq_bc[b][:],
                    )
                    tt = thp.tile([128, Q], F32, tag="tt", bufs=3,
                                  name=f"tt_{b}_{sb}_{t}")
                    nc.scalar.activation(tt[:], ta[:], tanh)
                    sc = thp.tile([128, Q], F32, tag="sc", bufs=3,
                                  name=f"sc_{b}_{sb}_{t}")
                    if not USE_TTR:
                        mul_eng = nc.gpsimd if GP_MUL else nc.vector
                        mul_eng.tensor_mul(sc[:], tt[:], we_bc[:])
                        nc.vector.tensor_reduce(
                            ecols[b][:, t : t + 1], sc[:],
                            axis=mybir.AxisListType.X, op=mybir.AluOpType.add,
                        )
                    else:
                        nc.vector.tensor_tensor_reduce(
                            out=sc[:],
                            in0=tt[:],
                            in1=we_bc[:],
                            scale=1.0,
                            scalar=0.0,
                            op0=mybir.AluOpType.mult,
                            op1=mybir.AluOpType.add,
                            accum_out=ecols[b][:, t : t + 1],
                        )

            # DVE rows: gather energies into rows of eraw via SBUF->SBUF DMA
            ecps = ecp.tile([ST, DVE_B * 128], F32, tag="ec",
                            name=f"ecp_{sb}")
            for b in range(PE_B, BL):
                i = b - PE_B
                nc.tensor.transpose(
                    ecps[:, i * 128 : (i + 1) * 128], ecols[b][:], ident[:]
                )
            ecT = thp.tile([ST, DVE_B * 128], F32, tag="ecT", bufs=2,
                           name=f"ecT_{sb}")
            nc.vector.tensor_copy(ecT[:], ecps[:])
            for i in range(DVE_B):
                nc.scalar.dma_start(
                    e_dve[i : i + 1, sb * SB : (sb + 1) * SB],
                    ecT[:, i * 128 : (i + 1) * 128],
                )

            em_fn, exp_fn = make_dve_finish(sb)
            if sb == NSB - 1:
                em_fn()
                exp_fn()
            else:
                pending = (em_fn, exp_fn)

            # PE rows: exp straight out of PSUM with fused z accumulation
            if not USE_ACCUM:
                nc.scalar.activation(
                    p_e[:, sb * SB : (sb + 1) * SB], e_ps[:], exp,
                )
                nc.vector.tensor_reduce(
                    z_pe_part[:, sb : sb + 1],
                    p_e[:, sb * SB : (sb + 1) * SB],
                    axis=mybir.AxisListType.X, op=mybir.AluOpType.add,
                )
            else:
                nc.scalar.activation(
                    p_e[:, sb * SB : (sb + 1) * SB], e_ps[:], exp,
                    accum_out=z_pe_part[:, sb : sb + 1],
                )

        # ---- finish softmax (per path: partition bases must be 0) ---------
        z_pe = outp.tile([PE_B, 1], F32)
        nc.vector.tensor_reduce(z_pe[:], z_pe_part[:],
                                axis=mybir.AxisListType.X,
                                op=mybir.AluOpType.add)
        zr_pe = outp.tile([PE_B, 1], F32)
        nc.vector.reciprocal(zr_pe[:], z_pe[:])
        z_dve = outp.tile([DVE_B, 1], F32)
        nc.vector.tensor_reduce(z_dve[:], z_dve_part[:],
                                axis=mybir.AxisListType.X,
                                op=mybir.AluOpType.add)
        zr_dve = outp.tile([DVE_B, 1], F32)
        nc.vector.reciprocal(zr_dve[:], z_dve[:])
        # reuse dead tiles as output staging: mask_i (PE rows), e_dve (DVE)
        a_pe = mask_i[0:PE_B, :].bitcast(F32)
        a_dve = e_dve
        for h in range(2):
            hs = S // 2
            cols = slice(h * hs, (h + 1) * hs)
            # PE rows scaled on scalar engine, DVE rows on vector — parallel
            if not USE_SMUL:
                nc.vector.tensor_scalar(
                    a_pe[:, cols], p_e[:, cols], zr_pe[:], None,
                    op0=mybir.AluOpType.mult,
                )
            else:
                nc.scalar.mul(a_pe[:, cols], p_e[:, cols], zr_pe[:])
            nc.sync.dma_start(attn_d[0:PE_B, cols], a_pe[:, cols])
            nc.vector.tensor_scalar(
                a_dve[:, cols], p_dve[:, cols], zr_dve[:], None,
                op0=mybir.AluOpType.mult,
            )
            nc.gpsimd.dma_start(attn_d[PE_B:BL, cols], a_dve[:, cols])

    nc.compile()
    return nc


def _get_nc():
    if "nc" not in _CACHE:
        _CACHE["nc"] = _build()
    return _CACHE["nc"]


def _make_in_maps(query, projected_memory, mask, Wq, We):
    query = np.asarray(query, dtype=np.float32)
    pm = np.asarray(projected_memory, dtype=np.float32)
    mask = np.asarray(mask, dtype=np.int32)
    wq = np.ascontiguousarray(np.asarray(Wq, dtype=np.float32))
    we = np.ascontiguousarray(np.asarray(We, dtype=np.float32))
    in_maps = []
    for i in range(N_CORES):
        lo, hi = i * BL, (i + 1) * BL
        in_maps.append(
            {
                "pm": np.ascontiguousarray(pm[lo:hi]),
                "q": np.ascontiguousarray(query[0, lo:hi, :]),
                "mask": np.ascontiguousarray(mask[lo:hi]),
                "wq": wq,
                "we": we,
            }
        )
    return in_maps


def run_spmd(query, projected_memory, mask, Wq, We, **spmd_kwargs):
    nc = _get_nc()
    in_maps = _make_in_maps(query, projected_memory, mask, Wq, We)
    return run_bass_kernel_spmd(nc, in_maps, list(range(N_CORES)), **spmd_kwargs)


def kernel(query, projected_memory, mask, Wq, We):
    res = run_spmd(query, projected_memory, mask, Wq, We)
    attn = np.concatenate([res.results[i]["attn"] for i in range(N_CORES)], axis=0)
    return attn[:, None, :].astype(np.float32)
